# revision 7
# baseline (speedup 1.0000x reference)
"""Bidirectional ConvLSTM encoder kernel for Trainium2 (Bass/Tile).

Problem: B=8, T=16, C=3, H=W=32, HID=64, 7x7 convs, bidirectional.
Sharding: data-parallel over batch; core b handles batch element b, running
both the forward and backward recurrences (2 independent recurrences that
ping-pong on the PE so gate/elementwise latency of one hides under the
other's matmuls).

Conv formulation: hidden 7x7 conv (64->256ch) is computed as a sum of
shifted matmuls over a zero-padded [64, 38, 38] state image. Taps are
packed in pairs onto the 128-deep contraction dim by keeping TWO copies of
the padded state: copy "A" holds (rows 0:64 = state, rows 64:128 = state
shifted down one row) pairing kernel rows (0,1),(2,3),(4,5); copy "B"
holds (rows 64:128 = state shifted right one col) pairing row-6 taps along
kw. 25 matmuls replace 49. The 3x7x7 input conv is im2col'd (K=147,
padded to 160 = 128+32) and accumulated into the same PSUM banks, so the
gate pre-activations g = W_ih*x + W_hh*h are formed entirely by the PE.
All matmul operands are fp16 (gates/cell state stay fp32; PSUM accumulates
fp32); walrus rejects mixed 32/16-bit matmul inputs and the fp32r
self-loading weight path trips a sync-wait-slot limit, so fp16 everywhere
on the PE it is.
"""

import numpy as np

HID = 64
T = 16
CIN = 3
H = 32
W = 32
HWSZ = H * W
PW = 38  # padded image width (32 + 2*3)
PAD = 3
KS = 7
NCORES = 8
KIN = CIN * KS * KS  # 147
KIN_PAD = 160  # 128 + 32 (zero-padded tail so the 2nd K-tile is a clean 32)

# Hidden-conv tap pairs: (kind, kh, kw).
#  "A": taps (kh, kw) + (kh+1, kw) via the row-shifted upper copy.
#  "B": taps (6, kw) + (6, kw+1) via the col-shifted upper copy.
#  "S": singleton tap (6, 6), K=64.
PAIRS = (
    [("A", kh0, kw) for kw in range(KS) for kh0 in (0, 2, 4)]
    + [("B", 6, kw0) for kw0 in (0, 2, 4)]
    + [("S", 6, 6)]
)
NPAIR = len(PAIRS)  # 25


def pack_whh(w_hh_f: np.ndarray, w_hh_b: np.ndarray) -> np.ndarray:
    """Pack hidden weights into lhsT tiles: [128(k), 2(dir), 25(pair), 2(mg), 128(m)].

    lhsT[k, d, p, mg, m] so that matmul(lhsT.T @ rhs) with rhs rows
    (k<64: tap_lo channel k, k>=64: tap_hi channel k-64) accumulates the conv.
    """
    out = np.zeros((2, NPAIR, 2, 128, 128), np.float32)  # d, p, mg, k, m
    for d, wsrc in enumerate([w_hh_f, w_hh_b]):
        wsrc = np.asarray(wsrc, dtype=np.float32)  # [256, 64, 7, 7]
        for p, (kind, r, c) in enumerate(PAIRS):
            if kind == "A":
                lo, hi = (r, c), (r + 1, c)
            elif kind == "B":
                lo, hi = (r, c), (r, c + 1)
            else:
                lo, hi = (r, c), None
            for mg in range(2):
                wm = wsrc[mg * 128 : (mg + 1) * 128]  # [128, 64, 7, 7]
                out[d, p, mg, 0:64, :] = wm[:, :, lo[0], lo[1]].T
                if hi is not None:
                    out[d, p, mg, 64:128, :] = wm[:, :, hi[0], hi[1]].T
    return np.ascontiguousarray(out.transpose(3, 0, 1, 2, 4).astype(np.float16))  # [k, d, p, mg, m]


def pack_wih(w_ih_f: np.ndarray, w_ih_b: np.ndarray) -> np.ndarray:
    """Pack input weights (im2col): [160(k), 2(dir), 2(mg), 128(m)]."""
    out = np.zeros((KIN_PAD, 2, 2, 128), np.float32)
    for d, wsrc in enumerate([w_ih_f, w_ih_b]):
        wk = np.asarray(wsrc, dtype=np.float32).reshape(256, KIN)  # (cin,kh,kw) C-order
        for mg in range(2):
            out[:KIN, d, mg, :] = wk[mg * 128 : (mg + 1) * 128].T
    return np.ascontiguousarray(out.astype(np.float16))


def pack_bias(b_ih_f, b_hh_f, b_ih_b, b_hh_b) -> np.ndarray:
    """[128(k), 2(dir), 2(mg)]: per-gate-channel bias."""
    out = np.zeros((128, 2, 2), np.float32)
    for d, (bi, bh) in enumerate([(b_ih_f, b_hh_f), (b_ih_b, b_hh_b)]):
        s = np.asarray(bi, dtype=np.float32) + np.asarray(bh, dtype=np.float32)  # [256]
        out[:, d, 0] = s[0:128]
        out[:, d, 1] = s[128:256]
    return np.ascontiguousarray(out)


def pack_xcol(xb: np.ndarray) -> np.ndarray:
    """im2col one batch element [T,3,32,32] -> [160(k), T, 2, 512]."""
    xb = np.asarray(xb, dtype=np.float32)
    xpad = np.pad(xb, ((0, 0), (0, 0), (PAD, PAD), (PAD, PAD)))
    win = np.lib.stride_tricks.sliding_window_view(xpad, (KS, KS), axis=(2, 3))
    # win: [T, 3, 32, 32, 7, 7] -> [(cin, kh, kw), T, hw]
    xcol = win.transpose(1, 4, 5, 0, 2, 3).reshape(KIN, T, HWSZ)
    out = np.zeros((KIN_PAD, T, 2, 512), np.float16)
    out[:KIN] = xcol.reshape(KIN, T, 2, 512).astype(np.float16)
    return out


def build_nc():
    import concourse.mybir as mybir
    from concourse import bacc
    from concourse.tile import TileContext

    F32 = mybir.dt.float32
    F32R = mybir.dt.float32r
    F16 = mybir.dt.float16
    AF = mybir.ActivationFunctionType

    nc = bacc.Bacc()
    xcol_d = nc.declare_dram_parameter("xcol", [KIN_PAD, T, 2, 512], F16, isOutput=False)
    whh_d = nc.declare_dram_parameter("whh", [128, 2, NPAIR, 2, 128], F16, isOutput=False)
    wih_d = nc.declare_dram_parameter("wih", [KIN_PAD, 2, 2, 128], F16, isOutput=False)
    bias_d = nc.declare_dram_parameter("bias", [128, 2, 2], F32, isOutput=False)
    out_d = nc.declare_dram_parameter("out", [T, 2, HID, H, W], F32, isOutput=True)

    with TileContext(nc) as tc:
        with (
            tc.tile_pool(name="wpool", bufs=1) as wpool,
            tc.tile_pool(name="state", bufs=1) as spool,
            tc.tile_pool(name="xin", bufs=2) as xpool,
            tc.tile_pool(name="work", bufs=1) as wkpool,
            tc.tile_pool(name="psum", bufs=1, space="PSUM") as pspool,
        ):
            whh = wpool.tile([128, 2, NPAIR, 2, 128], F16)
            nc.sync.dma_start(whh[:], whh_d[:])
            wih0 = wpool.tile([128, 2, 2, 128], F16)
            nc.sync.dma_start(wih0[:], wih_d[0:128])
            wih1 = wpool.tile([32, 2, 2, 128], F16)
            nc.sync.dma_start(wih1[:], wih_d[128:KIN_PAD])
            bias = wpool.tile([128, 2, 2], F32)
            nc.sync.dma_start(bias[:], bias_d[:])

            hA = [spool.tile([128, PW, PW], F16, tag=f"hA{d}", name=f"hA{d}") for d in range(2)]
            hB = [spool.tile([128, PW, PW], F16, tag=f"hB{d}", name=f"hB{d}") for d in range(2)]
            cst = [spool.tile([HID, H, W], F32, tag=f"c{d}", name=f"c{d}") for d in range(2)]
            for tl in (*hA, *hB, *cst):
                nc.vector.memset(tl[:], 0.0)

            for t in range(T):
                for d in range(2):
                    tsrc = t if d == 0 else T - 1 - t

                    xa = xpool.tile([128, 2, 512], F16, tag=f"xa{d}")
                    nc.sync.dma_start(xa[:], xcol_d[0:128, tsrc])
                    xb = xpool.tile([32, 2, 512], F16, tag=f"xb{d}")
                    nc.sync.dma_start(xb[:], xcol_d[128:KIN_PAD, tsrc])

                    ps0 = pspool.tile([128, 2, 512], F32, tag=f"ps{d}0")
                    ps1 = pspool.tile([128, 2, 512], F32, tag=f"ps{d}1")
                    pst = [ps0, ps1]

                    for mg in range(2):
                        # taps: list of (lhsT, rhs_fn(nh))
                        taps = [
                            (wih0[:, d, mg], lambda nh: xa[:, nh]),
                            (wih1[:, d, mg], lambda nh: xb[:, nh]),
                        ]
                        if t > 0:
                            for p, (kind, r, c) in enumerate(PAIRS):
                                if kind == "A":
                                    taps.append((
                                        whh[:, d, p, mg],
                                        lambda nh, r=r, c=c: hA[d][:, r + 16 * nh : r + 16 * nh + 16, c : c + 32],
                                    ))
                                elif kind == "B":
                                    taps.append((
                                        whh[:, d, p, mg],
                                        lambda nh, c=c: hB[d][:, 6 + 16 * nh : 6 + 16 * nh + 16, c : c + 32],
                                    ))
                                else:
                                    taps.append((
                                        whh[0:64, d, p, mg],
                                        lambda nh: hB[d][0:64, 6 + 16 * nh : 6 + 16 * nh + 16, 6:38],
                                    ))
                        n = len(taps)
                        for i, (lh, rhf) in enumerate(taps):
                            for nh in range(2):
                                nc.tensor.matmul(
                                    pst[mg][:, nh],
                                    lh,
                                    rhf(nh),
                                    start=(i == 0),
                                    stop=(i == n - 1),
                                )

                    sif = wkpool.tile([128, 2, 512], F32, tag=f"sif{d}")
                    sgo = wkpool.tile([128, 2, 512], F32, tag=f"sgo{d}")
                    tmp = wkpool.tile([HID, HWSZ], F32, tag=f"tmp{d}")
                    fd = wkpool.tile([HID, HWSZ], F32, tag=f"fd{d}")
                    od = wkpool.tile([HID, HWSZ], F32, tag=f"od{d}")
                    hcm = wkpool.tile([HID, H, W], F32, tag=f"h{d}")

                    # gates: i,f = sigmoid(mg0); g = tanh(mg1 lo); o = sigmoid(mg1 hi)
                    nc.scalar.activation(sif[:], ps0[:], AF.Sigmoid, bias=bias[:, d, 0:1])
                    nc.scalar.activation(sgo[0:64], ps1[0:64], AF.Tanh, bias=bias[0:64, d, 1:2])
                    nc.scalar.activation(sgo[64:128], ps1[64:128], AF.Sigmoid, bias=bias[64:128, d, 1:2])
                    # DVE/ACT are lane-locked: move f, o gates to partitions 0-63
                    nc.sync.dma_start(fd[:], sif[64:128])
                    nc.sync.dma_start(od[:], sgo[64:128])
                    # c = f*c + i*g ; h = o * tanh(c)
                    nc.vector.tensor_mul(tmp[:], sif[0:64], sgo[0:64])
                    nc.vector.tensor_mul(cst[d][:], cst[d][:], fd[:])
                    nc.vector.tensor_add(cst[d][:], cst[d][:], tmp[:])
                    nc.scalar.activation(sgo[0:64], cst[d][:], AF.Tanh)
                    nc.vector.tensor_mul(hcm[:], od[:], sgo[0:64])

                    nc.sync.dma_start(out_d[tsrc, d], hcm[:])
                    if t < T - 1:
                        # gpsimd (SWDGE) DMA casts fp32 -> fp16 into the state copies
                        nc.gpsimd.dma_start(hA[d][0:64, 3:35, 3:35], hcm[:])
                        nc.gpsimd.dma_start(hA[d][64:128, 2:34, 3:35], hcm[:])
                        nc.gpsimd.dma_start(hB[d][0:64, 3:35, 3:35], hcm[:])
                        nc.gpsimd.dma_start(hB[d][64:128, 3:35, 2:34], hcm[:])
    nc.compile()
    return nc


_CACHE = {}


def get_nc():
    if "nc" not in _CACHE:
        _CACHE["nc"] = build_nc()
    return _CACHE["nc"]


def make_in_maps(inputs):
    shared = {
        "whh": pack_whh(inputs["w_hh_f"], inputs["w_hh_b"]),
        "wih": pack_wih(inputs["w_ih_f"], inputs["w_ih_b"]),
        "bias": pack_bias(
            inputs["b_ih_f"], inputs["b_hh_f"], inputs["b_ih_b"], inputs["b_hh_b"]
        ),
    }
    x = np.asarray(inputs["x"], dtype=np.float32)
    return [dict(shared, xcol=pack_xcol(x[b])) for b in range(NCORES)]


def assemble(results):
    final = np.empty((NCORES, T, 2 * HID, H, W), np.float32)
    for b in range(NCORES):
        ob = results[b]["out"]  # [T, 2, HID, H, W]
        final[b, :, 0:HID] = ob[:, 0]
        final[b, :, HID:] = ob[:, 1]
    return final


def run_on_device(inputs, **kwargs):
    from concourse.bass_utils import run_bass_kernel_spmd

    nc = get_nc()
    in_maps = make_in_maps(inputs)
    res = run_bass_kernel_spmd(nc, in_maps, core_ids=list(range(NCORES)), **kwargs)
    return assemble(res.results), res


def kernel(**inputs):
    out, _ = run_on_device(inputs)
    return out


# revision 8
# speedup vs baseline: 1.0320x; 1.0320x over previous
"""Bidirectional ConvLSTM encoder kernel for Trainium2 (Bass/Tile).

Problem: B=8, T=16, C=3, H=W=32, HID=64, 7x7 convs, bidirectional.
Sharding: data-parallel over batch; core b handles batch element b, running
both the forward and backward recurrences (2 independent recurrences that
ping-pong on the PE so gate/elementwise latency of one hides under the
other's matmuls).

Conv formulation: hidden 7x7 conv (64->256ch) is computed as a sum of
shifted matmuls over a zero-padded [64, 38, 38] state image. Taps are
packed in pairs onto the 128-deep contraction dim by keeping TWO copies of
the padded state: copy "A" holds (rows 0:64 = state, rows 64:128 = state
shifted down one row) pairing kernel rows (0,1),(2,3),(4,5); copy "B"
holds (rows 64:128 = state shifted right one col) pairing row-6 taps along
kw. 25 matmuls replace 49. The 3x7x7 input conv is im2col'd (K=147,
padded to 160 = 128+32) and accumulated into the same PSUM banks, so the
gate pre-activations g = W_ih*x + W_hh*h are formed entirely by the PE.
All matmul operands are fp16 (gates/cell state stay fp32; PSUM accumulates
fp32); walrus rejects mixed 32/16-bit matmul inputs and the fp32r
self-loading weight path trips a sync-wait-slot limit, so fp16 everywhere
on the PE it is.
"""

import numpy as np

HID = 64
T = 16
CIN = 3
H = 32
W = 32
HWSZ = H * W
PW = 38  # padded image width (32 + 2*3)
PAD = 3
KS = 7
NCORES = 8
KIN = CIN * KS * KS  # 147
KIN_PAD = 160  # 128 + 32 (zero-padded tail so the 2nd K-tile is a clean 32)

# Hidden-conv tap pairs: (kind, kh, kw).
#  "A": taps (kh, kw) + (kh+1, kw) via the row-shifted upper copy.
#  "B": taps (6, kw) + (6, kw+1) via the col-shifted upper copy.
#  "S": singleton tap (6, 6), K=64.
PAIRS = (
    [("A", kh0, kw) for kw in range(KS) for kh0 in (0, 2, 4)]
    + [("B", 6, kw0) for kw0 in (0, 2, 4)]
    + [("S", 6, 6)]
)
NPAIR = len(PAIRS)  # 25


def pack_whh(w_hh_f: np.ndarray, w_hh_b: np.ndarray) -> np.ndarray:
    """Pack hidden weights into lhsT tiles: [128(k), 2(dir), 25(pair), 2(mg), 128(m)].

    lhsT[k, d, p, mg, m] so that matmul(lhsT.T @ rhs) with rhs rows
    (k<64: tap_lo channel k, k>=64: tap_hi channel k-64) accumulates the conv.
    """
    out = np.zeros((2, NPAIR, 2, 128, 128), np.float32)  # d, p, mg, k, m
    for d, wsrc in enumerate([w_hh_f, w_hh_b]):
        wsrc = np.asarray(wsrc, dtype=np.float32)  # [256, 64, 7, 7]
        for p, (kind, r, c) in enumerate(PAIRS):
            if kind == "A":
                lo, hi = (r, c), (r + 1, c)
            elif kind == "B":
                lo, hi = (r, c), (r, c + 1)
            else:
                lo, hi = (r, c), None
            for mg in range(2):
                wm = wsrc[mg * 128 : (mg + 1) * 128]  # [128, 64, 7, 7]
                out[d, p, mg, 0:64, :] = wm[:, :, lo[0], lo[1]].T
                if hi is not None:
                    out[d, p, mg, 64:128, :] = wm[:, :, hi[0], hi[1]].T
    return np.ascontiguousarray(out.transpose(3, 0, 1, 2, 4).astype(np.float16))  # [k, d, p, mg, m]


def pack_wih(w_ih_f: np.ndarray, w_ih_b: np.ndarray) -> np.ndarray:
    """Pack input weights (im2col): [160(k), 2(dir), 2(mg), 128(m)]."""
    out = np.zeros((KIN_PAD, 2, 2, 128), np.float32)
    for d, wsrc in enumerate([w_ih_f, w_ih_b]):
        wk = np.asarray(wsrc, dtype=np.float32).reshape(256, KIN)  # (cin,kh,kw) C-order
        for mg in range(2):
            out[:KIN, d, mg, :] = wk[mg * 128 : (mg + 1) * 128].T
    return np.ascontiguousarray(out.astype(np.float16))


def pack_bias(b_ih_f, b_hh_f, b_ih_b, b_hh_b) -> np.ndarray:
    """[128(k), 2(dir), 2(mg)]: per-gate-channel bias."""
    out = np.zeros((128, 2, 2), np.float32)
    for d, (bi, bh) in enumerate([(b_ih_f, b_hh_f), (b_ih_b, b_hh_b)]):
        s = np.asarray(bi, dtype=np.float32) + np.asarray(bh, dtype=np.float32)  # [256]
        out[:, d, 0] = s[0:128]
        out[:, d, 1] = s[128:256]
    return np.ascontiguousarray(out)


def pack_xcol(xb: np.ndarray) -> np.ndarray:
    """im2col one batch element [T,3,32,32] -> [160(k), T, 2, 512]."""
    xb = np.asarray(xb, dtype=np.float32)
    xpad = np.pad(xb, ((0, 0), (0, 0), (PAD, PAD), (PAD, PAD)))
    win = np.lib.stride_tricks.sliding_window_view(xpad, (KS, KS), axis=(2, 3))
    # win: [T, 3, 32, 32, 7, 7] -> [(cin, kh, kw), T, hw]
    xcol = win.transpose(1, 4, 5, 0, 2, 3).reshape(KIN, T, HWSZ)
    out = np.zeros((KIN_PAD, T, 2, 512), np.float16)
    out[:KIN] = xcol.reshape(KIN, T, 2, 512).astype(np.float16)
    return out


def build_nc():
    import concourse.mybir as mybir
    from concourse import bacc
    from concourse.tile import TileContext

    F32 = mybir.dt.float32
    F32R = mybir.dt.float32r
    F16 = mybir.dt.float16
    AF = mybir.ActivationFunctionType

    nc = bacc.Bacc()
    xcol_d = nc.declare_dram_parameter("xcol", [KIN_PAD, T, 2, 512], F16, isOutput=False)
    whh_d = nc.declare_dram_parameter("whh", [128, 2, NPAIR, 2, 128], F16, isOutput=False)
    wih_d = nc.declare_dram_parameter("wih", [KIN_PAD, 2, 2, 128], F16, isOutput=False)
    bias_d = nc.declare_dram_parameter("bias", [128, 2, 2], F32, isOutput=False)
    out_d = nc.declare_dram_parameter("out", [T, 2, HID, H, W], F32, isOutput=True)

    with TileContext(nc) as tc:
        with (
            tc.tile_pool(name="wpool", bufs=1) as wpool,
            tc.tile_pool(name="state", bufs=1) as spool,
            tc.tile_pool(name="xin", bufs=2) as xpool,
            tc.tile_pool(name="work", bufs=1) as wkpool,
            tc.tile_pool(name="psum", bufs=1, space="PSUM") as pspool,
        ):
            whh = wpool.tile([128, 2, NPAIR, 2, 128], F16)
            nc.sync.dma_start(whh[:], whh_d[:])
            wih0 = wpool.tile([128, 2, 2, 128], F16)
            nc.sync.dma_start(wih0[:], wih_d[0:128])
            wih1 = wpool.tile([32, 2, 2, 128], F16)
            nc.sync.dma_start(wih1[:], wih_d[128:KIN_PAD])
            bias = wpool.tile([128, 2, 2], F32)
            nc.sync.dma_start(bias[:], bias_d[:])

            hA = [spool.tile([128, PW, PW], F16, tag=f"hA{d}", name=f"hA{d}") for d in range(2)]
            hB = [spool.tile([128, PW, PW], F16, tag=f"hB{d}", name=f"hB{d}") for d in range(2)]
            cst = [spool.tile([HID, H, W], F32, tag=f"c{d}", name=f"c{d}") for d in range(2)]
            for tl in (*hA, *hB, *cst):
                nc.vector.memset(tl[:], 0.0)

            for t in range(T):
                for d in range(2):
                    tsrc = t if d == 0 else T - 1 - t

                    xa = xpool.tile([128, 2, 512], F16, tag=f"xa{d}")
                    nc.sync.dma_start(xa[:], xcol_d[0:128, tsrc])
                    xb = xpool.tile([32, 2, 512], F16, tag=f"xb{d}")
                    nc.sync.dma_start(xb[:], xcol_d[128:KIN_PAD, tsrc])

                    ps0 = pspool.tile([128, 2, 512], F32, tag=f"ps{d}0")
                    ps1 = pspool.tile([128, 2, 512], F32, tag=f"ps{d}1")
                    pst = [ps0, ps1]

                    for mg in range(2):
                        # taps: list of (lhsT, rhs_fn(nh))
                        taps = [
                            (wih0[:, d, mg], lambda nh: xa[:, nh]),
                            (wih1[:, d, mg], lambda nh: xb[:, nh]),
                        ]
                        if t > 0:
                            for p, (kind, r, c) in enumerate(PAIRS):
                                if kind == "A":
                                    taps.append((
                                        whh[:, d, p, mg],
                                        lambda nh, r=r, c=c: hA[d][:, r + 16 * nh : r + 16 * nh + 16, c : c + 32],
                                    ))
                                elif kind == "B":
                                    taps.append((
                                        whh[:, d, p, mg],
                                        lambda nh, c=c: hB[d][:, 6 + 16 * nh : 6 + 16 * nh + 16, c : c + 32],
                                    ))
                                else:
                                    taps.append((
                                        whh[0:64, d, p, mg],
                                        lambda nh: hB[d][0:64, 6 + 16 * nh : 6 + 16 * nh + 16, 6:38],
                                    ))
                        n = len(taps)
                        for i, (lh, rhf) in enumerate(taps):
                            for nh in range(2):
                                nc.tensor.matmul(
                                    pst[mg][:, nh],
                                    lh,
                                    rhf(nh),
                                    start=(i == 0),
                                    stop=(i == n - 1),
                                )

                    sif = wkpool.tile([128, 2, 512], F32, tag=f"sif{d}")
                    sgo = wkpool.tile([128, 2, 512], F32, tag=f"sgo{d}")
                    tmp = wkpool.tile([HID, HWSZ], F32, tag=f"tmp{d}")
                    fd = wkpool.tile([HID, HWSZ], F32, tag=f"fd{d}")
                    od = wkpool.tile([HID, HWSZ], F32, tag=f"od{d}")
                    hcm = wkpool.tile([HID, H, W], F32, tag=f"h{d}")
                    hcm16 = wkpool.tile([HID, H, W], F16, tag=f"h16{d}")

                    # gates: i,f = sigmoid(mg0); g = tanh(mg1 lo); o = sigmoid(mg1 hi)
                    nc.scalar.activation(sif[:], ps0[:], AF.Sigmoid, bias=bias[:, d, 0:1])
                    nc.scalar.activation(sgo[0:64], ps1[0:64], AF.Tanh, bias=bias[0:64, d, 1:2])
                    nc.scalar.activation(sgo[64:128], ps1[64:128], AF.Sigmoid, bias=bias[64:128, d, 1:2])
                    # DVE/ACT are lane-locked: move f, o gates to partitions 0-63
                    nc.sync.dma_start(fd[:], sif[64:128])
                    nc.sync.dma_start(od[:], sgo[64:128])
                    # c = f*c + i*g ; h = o * tanh(c)
                    nc.vector.tensor_mul(tmp[:], sif[0:64], sgo[0:64])
                    nc.vector.tensor_mul(cst[d][:], cst[d][:], fd[:])
                    nc.vector.tensor_add(cst[d][:], cst[d][:], tmp[:])
                    nc.scalar.activation(sgo[0:64], cst[d][:], AF.Tanh)
                    nc.vector.tensor_mul(hcm[:], od[:], sgo[0:64])

                    nc.sync.dma_start(out_d[tsrc, d], hcm[:])
                    if t < T - 1:
                        # fp16 copy of h for the state images (HWDGE same-dtype DMAs)
                        nc.vector.tensor_mul(hcm16[:], od[:], sgo[0:64])
                        nc.sync.dma_start(hA[d][0:64, 3:35, 3:35], hcm16[:])
                        nc.sync.dma_start(hA[d][64:128, 2:34, 3:35], hcm16[:])
                        nc.sync.dma_start(hB[d][0:64, 3:35, 3:35], hcm16[:])
                        nc.sync.dma_start(hB[d][64:128, 3:35, 2:34], hcm16[:])
    nc.compile()
    return nc


_CACHE = {}


def get_nc():
    if "nc" not in _CACHE:
        _CACHE["nc"] = build_nc()
    return _CACHE["nc"]


def make_in_maps(inputs):
    shared = {
        "whh": pack_whh(inputs["w_hh_f"], inputs["w_hh_b"]),
        "wih": pack_wih(inputs["w_ih_f"], inputs["w_ih_b"]),
        "bias": pack_bias(
            inputs["b_ih_f"], inputs["b_hh_f"], inputs["b_ih_b"], inputs["b_hh_b"]
        ),
    }
    x = np.asarray(inputs["x"], dtype=np.float32)
    return [dict(shared, xcol=pack_xcol(x[b])) for b in range(NCORES)]


def assemble(results):
    final = np.empty((NCORES, T, 2 * HID, H, W), np.float32)
    for b in range(NCORES):
        ob = results[b]["out"]  # [T, 2, HID, H, W]
        final[b, :, 0:HID] = ob[:, 0]
        final[b, :, HID:] = ob[:, 1]
    return final


def run_on_device(inputs, **kwargs):
    from concourse.bass_utils import run_bass_kernel_spmd

    nc = get_nc()
    in_maps = make_in_maps(inputs)
    res = run_bass_kernel_spmd(nc, in_maps, core_ids=list(range(NCORES)), **kwargs)
    return assemble(res.results), res


def kernel(**inputs):
    out, _ = run_on_device(inputs)
    return out


# revision 9
# speedup vs baseline: 1.0648x; 1.0318x over previous
"""Bidirectional ConvLSTM encoder kernel for Trainium2 (Bass/Tile).

Problem: B=8, T=16, C=3, H=W=32, HID=64, 7x7 convs, bidirectional.
Sharding: data-parallel over batch; core b handles batch element b, running
both the forward and backward recurrences (2 independent recurrences that
ping-pong on the PE so gate/elementwise latency of one hides under the
other's matmuls).

Conv formulation: hidden 7x7 conv (64->256ch) is computed as a sum of
shifted matmuls over a zero-padded [64, 38, 38] state image. Taps are
packed in pairs onto the 128-deep contraction dim by keeping TWO copies of
the padded state: copy "A" holds (rows 0:64 = state, rows 64:128 = state
shifted down one row) pairing kernel rows (0,1),(2,3),(4,5); copy "B"
holds (rows 64:128 = state shifted right one col) pairing row-6 taps along
kw. 25 matmuls replace 49. The 3x7x7 input conv is im2col'd (K=147,
padded to 160 = 128+32) and accumulated into the same PSUM banks, so the
gate pre-activations g = W_ih*x + W_hh*h are formed entirely by the PE.
All matmul operands are fp16 (gates/cell state stay fp32; PSUM accumulates
fp32); walrus rejects mixed 32/16-bit matmul inputs and the fp32r
self-loading weight path trips a sync-wait-slot limit, so fp16 everywhere
on the PE it is.
"""

import numpy as np

HID = 64
T = 16
CIN = 3
H = 32
W = 32
HWSZ = H * W
PW = 38  # padded image width (32 + 2*3)
PAD = 3
KS = 7
NCORES = 8
KIN = CIN * KS * KS  # 147
KIN_PAD = 160  # 128 + 32 (zero-padded tail so the 2nd K-tile is a clean 32)

# Hidden-conv tap pairs: (kind, kh, kw).
#  "A": taps (kh, kw) + (kh+1, kw) via the row-shifted upper copy.
#  "B": taps (6, kw) + (6, kw+1) via the col-shifted upper copy.
#  "S": singleton tap (6, 6), K=64.
PAIRS = (
    [("A", kh0, kw) for kw in range(KS) for kh0 in (0, 2, 4)]
    + [("B", 6, kw0) for kw0 in (0, 2, 4)]
    + [("S", 6, 6)]
)
NPAIR = len(PAIRS)  # 25


def pack_whh(w_hh_f: np.ndarray, w_hh_b: np.ndarray) -> np.ndarray:
    """Pack hidden weights into lhsT tiles: [128(k), 2(dir), 25(pair), 2(mg), 128(m)].

    lhsT[k, d, p, mg, m] so that matmul(lhsT.T @ rhs) with rhs rows
    (k<64: tap_lo channel k, k>=64: tap_hi channel k-64) accumulates the conv.
    """
    out = np.zeros((2, NPAIR, 2, 128, 128), np.float32)  # d, p, mg, k, m
    for d, wsrc in enumerate([w_hh_f, w_hh_b]):
        wsrc = np.asarray(wsrc, dtype=np.float32)  # [256, 64, 7, 7]
        for p, (kind, r, c) in enumerate(PAIRS):
            if kind == "A":
                lo, hi = (r, c), (r + 1, c)
            elif kind == "B":
                lo, hi = (r, c), (r, c + 1)
            else:
                lo, hi = (r, c), None
            for mg in range(2):
                wm = wsrc[mg * 128 : (mg + 1) * 128]  # [128, 64, 7, 7]
                out[d, p, mg, 0:64, :] = wm[:, :, lo[0], lo[1]].T
                if hi is not None:
                    out[d, p, mg, 64:128, :] = wm[:, :, hi[0], hi[1]].T
    return np.ascontiguousarray(out.transpose(3, 0, 1, 2, 4).astype(np.float16))  # [k, d, p, mg, m]


def pack_wih(w_ih_f: np.ndarray, w_ih_b: np.ndarray) -> np.ndarray:
    """Pack input weights (im2col): [160(k), 2(dir), 2(mg), 128(m)]."""
    out = np.zeros((KIN_PAD, 2, 2, 128), np.float32)
    for d, wsrc in enumerate([w_ih_f, w_ih_b]):
        wk = np.asarray(wsrc, dtype=np.float32).reshape(256, KIN)  # (cin,kh,kw) C-order
        for mg in range(2):
            out[:KIN, d, mg, :] = wk[mg * 128 : (mg + 1) * 128].T
    return np.ascontiguousarray(out.astype(np.float16))


def pack_bias(b_ih_f, b_hh_f, b_ih_b, b_hh_b) -> np.ndarray:
    """[128(k), 2(dir), 2(mg)]: per-gate-channel bias."""
    out = np.zeros((128, 2, 2), np.float32)
    for d, (bi, bh) in enumerate([(b_ih_f, b_hh_f), (b_ih_b, b_hh_b)]):
        s = np.asarray(bi, dtype=np.float32) + np.asarray(bh, dtype=np.float32)  # [256]
        out[:, d, 0] = s[0:128]
        out[:, d, 1] = s[128:256]
    return np.ascontiguousarray(out)


def pack_xcol(xb: np.ndarray) -> np.ndarray:
    """im2col one batch element [T,3,32,32] -> [160(k), T, 2, 512]."""
    xb = np.asarray(xb, dtype=np.float32)
    xpad = np.pad(xb, ((0, 0), (0, 0), (PAD, PAD), (PAD, PAD)))
    win = np.lib.stride_tricks.sliding_window_view(xpad, (KS, KS), axis=(2, 3))
    # win: [T, 3, 32, 32, 7, 7] -> [(cin, kh, kw), T, hw]
    xcol = win.transpose(1, 4, 5, 0, 2, 3).reshape(KIN, T, HWSZ)
    out = np.zeros((KIN_PAD, T, 2, 512), np.float16)
    out[:KIN] = xcol.reshape(KIN, T, 2, 512).astype(np.float16)
    return out


def build_nc():
    import concourse.mybir as mybir
    from concourse import bacc
    from concourse.tile import TileContext

    F32 = mybir.dt.float32
    F32R = mybir.dt.float32r
    F16 = mybir.dt.float16
    AF = mybir.ActivationFunctionType

    nc = bacc.Bacc()
    xcol_d = nc.declare_dram_parameter("xcol", [KIN_PAD, T, 2, 512], F16, isOutput=False)
    whh_d = nc.declare_dram_parameter("whh", [128, 2, NPAIR, 2, 128], F16, isOutput=False)
    wih_d = nc.declare_dram_parameter("wih", [KIN_PAD, 2, 2, 128], F16, isOutput=False)
    bias_d = nc.declare_dram_parameter("bias", [128, 2, 2], F32, isOutput=False)
    out_d = nc.declare_dram_parameter("out", [T, 2, HID, H, W], F32, isOutput=True)

    with TileContext(nc) as tc:
        with (
            tc.tile_pool(name="wpool", bufs=1) as wpool,
            tc.tile_pool(name="state", bufs=1) as spool,
            tc.tile_pool(name="xin", bufs=2) as xpool,
            tc.tile_pool(name="work", bufs=1) as wkpool,
            tc.tile_pool(name="psum", bufs=1, space="PSUM") as pspool,
        ):
            whh = wpool.tile([128, 2, NPAIR, 2, 128], F16)
            wih0 = wpool.tile([128, 2, 2, 128], F16)
            nc.sync.dma_start(wih0[:], wih_d[0:128])
            wih1 = wpool.tile([32, 2, 2, 128], F16)
            nc.sync.dma_start(wih1[:], wih_d[128:KIN_PAD])
            bias = wpool.tile([128, 2, 2], F32)
            nc.sync.dma_start(bias[:], bias_d[:])

            hAB = [spool.tile([128, 2, PW, PW], F16, tag=f"hAB{d}", name=f"hAB{d}") for d in range(2)]
            cst = [spool.tile([HID, H, W], F32, tag=f"c{d}", name=f"c{d}") for d in range(2)]
            for tl in (*hAB, *cst):
                nc.vector.memset(tl[:], 0.0)

            for t in range(T):
                if t == 1:
                    # deferred so t=0's input DMAs get the DMA queues first
                    nc.sync.dma_start(whh[:, 0], whh_d[:, 0])
                    nc.sync.dma_start(whh[:, 1], whh_d[:, 1])
                for d in range(2):
                    tsrc = t if d == 0 else T - 1 - t

                    xa = xpool.tile([128, 2, 512], F16, tag=f"xa{d}")
                    nc.sync.dma_start(xa[:], xcol_d[0:128, tsrc])
                    xb = xpool.tile([32, 2, 512], F16, tag=f"xb{d}")
                    nc.sync.dma_start(xb[:], xcol_d[128:KIN_PAD, tsrc])

                    ps0 = pspool.tile([128, 2, 512], F32, tag=f"ps{d}0")
                    ps1 = pspool.tile([128, 2, 512], F32, tag=f"ps{d}1")
                    pst = [ps0, ps1]

                    for mg in range(2):
                        # taps: list of (lhsT, rhs_fn(nh))
                        taps = [
                            (wih0[:, d, mg], lambda nh: xa[:, nh]),
                            (wih1[:, d, mg], lambda nh: xb[:, nh]),
                        ]
                        if t > 0:
                            for p, (kind, r, c) in enumerate(PAIRS):
                                if kind == "A":
                                    taps.append((
                                        whh[:, d, p, mg],
                                        lambda nh, r=r, c=c: hAB[d][:, 0, r + 16 * nh : r + 16 * nh + 16, c : c + 32],
                                    ))
                                elif kind == "B":
                                    taps.append((
                                        whh[:, d, p, mg],
                                        lambda nh, c=c: hAB[d][:, 1, 6 + 16 * nh : 6 + 16 * nh + 16, c : c + 32],
                                    ))
                                else:
                                    taps.append((
                                        whh[0:64, d, p, mg],
                                        lambda nh: hAB[d][0:64, 1, 6 + 16 * nh : 6 + 16 * nh + 16, 6:38],
                                    ))
                        n = len(taps)
                        for i, (lh, rhf) in enumerate(taps):
                            for nh in range(2):
                                nc.tensor.matmul(
                                    pst[mg][:, nh],
                                    lh,
                                    rhf(nh),
                                    start=(i == 0),
                                    stop=(i == n - 1),
                                )

                    sif = wkpool.tile([128, 2, 512], F32, tag=f"sif{d}")
                    sgo = wkpool.tile([128, 2, 512], F32, tag=f"sgo{d}")
                    tmp = wkpool.tile([HID, HWSZ], F32, tag=f"tmp{d}")
                    fd = wkpool.tile([HID, HWSZ], F32, tag=f"fd{d}")
                    od = wkpool.tile([HID, HWSZ], F32, tag=f"od{d}")
                    hcm = wkpool.tile([HID, H, W], F32, tag=f"h{d}")

                    # gates: i,f = sigmoid(mg0); g = tanh(mg1 lo); o = sigmoid(mg1 hi)
                    nc.scalar.activation(sif[:], ps0[:], AF.Sigmoid, bias=bias[:, d, 0:1])
                    nc.scalar.activation(sgo[0:64], ps1[0:64], AF.Tanh, bias=bias[0:64, d, 1:2])
                    nc.scalar.activation(sgo[64:128], ps1[64:128], AF.Sigmoid, bias=bias[64:128, d, 1:2])
                    # lane-locked engines: move f, o gates to partitions 0-63 via
                    # the ACT engine's own DGE ring (no cross-engine sem needed)
                    if t > 0:
                        nc.scalar.dma_start(fd[:], sif[64:128])
                    nc.scalar.dma_start(od[:], sgo[64:128])
                    # c = f*c + i*g ; h = o * tanh(c)
                    th = sgo[0:64]
                    if t > 0:
                        nc.vector.tensor_mul(tmp[:], sif[0:64], th)
                        nc.vector.tensor_mul(cst[d][:], cst[d][:], fd[:])
                        nc.vector.tensor_add(cst[d][:], cst[d][:], tmp[:])
                    else:
                        nc.vector.tensor_mul(cst[d][:], sif[0:64], th)
                    nc.scalar.activation(th, cst[d][:], AF.Tanh)
                    nc.vector.tensor_mul(hcm[:], od[:], th)

                    nc.scalar.dma_start(out_d[tsrc, d], hcm[:])
                    if t < T - 1:
                        # write both lower state copies in one broadcast DVE op (fp16)
                        od3 = od[:].rearrange("p (a b) -> p a b", a=H).unsqueeze(1).to_broadcast([HID, 2, H, W])
                        th3 = th.rearrange("p a b -> p (a b)").rearrange("p (a b) -> p a b", a=H).unsqueeze(1).to_broadcast([HID, 2, H, W])
                        nc.vector.tensor_mul(hAB[d][0:64, :, 3:35, 3:35], od3, th3)
                        # upper (shifted) copies: strided DMAs, split across two DGE rings
                        hlow = hAB[d][0:64, 0, 3:35, 3:35]
                        nc.sync.dma_start(hAB[d][64:128, 0, 2:34, 3:35], hlow)
                        nc.scalar.dma_start(hAB[d][64:128, 1, 3:35, 2:34], hlow)
    nc.compile()
    return nc


_CACHE = {}


def get_nc():
    if "nc" not in _CACHE:
        _CACHE["nc"] = build_nc()
    return _CACHE["nc"]


def make_in_maps(inputs):
    shared = {
        "whh": pack_whh(inputs["w_hh_f"], inputs["w_hh_b"]),
        "wih": pack_wih(inputs["w_ih_f"], inputs["w_ih_b"]),
        "bias": pack_bias(
            inputs["b_ih_f"], inputs["b_hh_f"], inputs["b_ih_b"], inputs["b_hh_b"]
        ),
    }
    x = np.asarray(inputs["x"], dtype=np.float32)
    return [dict(shared, xcol=pack_xcol(x[b])) for b in range(NCORES)]


def assemble(results):
    final = np.empty((NCORES, T, 2 * HID, H, W), np.float32)
    for b in range(NCORES):
        ob = results[b]["out"]  # [T, 2, HID, H, W]
        final[b, :, 0:HID] = ob[:, 0]
        final[b, :, HID:] = ob[:, 1]
    return final


def run_on_device(inputs, **kwargs):
    from concourse.bass_utils import run_bass_kernel_spmd

    nc = get_nc()
    in_maps = make_in_maps(inputs)
    res = run_bass_kernel_spmd(nc, in_maps, core_ids=list(range(NCORES)), **kwargs)
    return assemble(res.results), res


def kernel(**inputs):
    out, _ = run_on_device(inputs)
    return out
